# revision 6
# baseline (speedup 1.0000x reference)
"""Bass/Trainium2 kernel for nn_D_constraint1: 0.001*sqrt(sum_i (||d_i||^2 - 1)^2).

Sharding: d [16384, 2048] is split row-wise across 8 NeuronCores (2048 rows
each). Each core computes sum over its rows of (||row||^2 - 1)^2, reduced to a
[128, 1] per-partition partial. The host gathers the 8x128 partials, sums,
takes sqrt and scales — the scalar "all-reduce" of the sharding hint.
"""

import numpy as np

import concourse.bass as bass
import concourse.tile as tile
from concourse import bacc, mybir
from concourse.bass_utils import run_bass_kernel_spmd

N, K = 16384, 2048
NCORES = 8
R = N // NCORES  # rows per core
P = 128          # SBUF partitions
T = R // P       # row-tiles per core

_nc_cache = None


def _build_nc():
    f32 = mybir.dt.float32
    nc = bacc.Bacc("TRN2", target_bir_lowering=False, debug=False)
    d = nc.dram_tensor("d", [R, K], f32, kind="ExternalInput").ap()
    out = nc.dram_tensor("out", [P, 1], f32, kind="ExternalOutput").ap()
    Square = mybir.ActivationFunctionType.Square

    with tile.TileContext(nc) as tc:
        with (
            tc.tile_pool(name="inp", bufs=4) as inp,
            tc.tile_pool(name="sq", bufs=2) as sqp,
            tc.tile_pool(name="stat", bufs=1) as stat,
        ):
            s = stat.tile([P, T], f32)  # per-row ||row||^2, one column per tile
            neg1 = stat.tile([P, 1], f32)
            nc.gpsimd.memset(neg1[:], -1.0)
            for i in range(T):
                t = inp.tile([P, K], f32)
                nc.sync.dma_start(t[:], d[i * P : (i + 1) * P, :])
                junk = sqp.tile([P, K], f32)
                nc.scalar.activation(junk[:], t[:], Square, accum_out=s[:, i : i + 1])
            junk2 = stat.tile([P, T], f32)
            partial = stat.tile([P, 1], f32)
            # partial[p] = sum_i (s[p,i] - 1)^2
            nc.scalar.activation(
                junk2[:], s[:], Square, bias=neg1[:], scale=1.0, accum_out=partial[:]
            )
            nc.sync.dma_start(out, partial[:])
    nc.compile()
    return nc


def _get_nc():
    global _nc_cache
    if _nc_cache is None:
        _nc_cache = _build_nc()
    return _nc_cache


def run_shards(d, **spmd_kwargs):
    """Run the SPMD kernel; returns the BassKernelResults (for profiling)."""
    d = np.ascontiguousarray(np.asarray(d, dtype=np.float32))
    assert d.shape == (N, K), d.shape
    shards = d.reshape(NCORES, R, K)
    in_maps = [{"d": shards[c]} for c in range(NCORES)]
    return run_bass_kernel_spmd(_get_nc(), in_maps, list(range(NCORES)), **spmd_kwargs)

def _combine(results):
    total = 0.0
    for r in results:
        total += np.sum(r["out"].astype(np.float64))
    return np.float32(0.001 * np.sqrt(total))


def kernel(d):
    return _combine(run_shards(d).results)
